# revision 21
# baseline (speedup 1.0000x reference)
"""ContraCLM token-level contrastive loss on 8 Trainium2 NeuronCores.

Data-parallel over the batch: core b handles sample b (B=8). Per core,
with S=1536, D=1024, T=0.05:

  The host supplies each view pre-transposed (hT = h.T, [D, S] fp32), a
  row-broadcast masked scale helper m8b = broadcast(8*mask) [128, S],
  and the token-major mask maskT [128, S/128]. On device, per view:

    sq   = hT * hT                      (GpSimd, bf16 out)
    nrow = ones^T @ sq                  (PE column sums -> |h_t|^2, [1,S])
    srow = exp(-0.5 ln nrow)            (ScalarE, natural_log_exp set)
    sb   = partition_broadcast(srow)    (GpSimd)
    smask= sb * m8b                     (DVE; 8/||h_t|| * mask)
    fT   = hT * smask                   (GpSimd, fp8e4 out, x8 scaled)

  No on-device transposes: fT is built directly in [D, 2S] layout.

  sim = F F^T as [128, 512] PSUM strips (fp8 DoubleRow, K=1024).
  exp(sim/T) row sums come free from the ScalarE activation accumulator.
  A and D quadrants are symmetric: only rows r <= 4cs+3 of each column
  strip are computed; the skipped blocks' row sums are recovered as
  PSUM-accumulated ones-matmul column sums of their mirrors (caccA/
  caccD), like the C quadrant reuses B's column sums (cacc).

  Self-similarity diagonal blocks are zeroed (affine_select + DVE row
  sum) before summing: exp(1/T) = e^20 would destroy the fp32 sum. The
  positive-counterpart diagonal (B quadrant) is LEFT IN the row sum:
  denom = Ng + pos and the included diagonal IS exp(pos_sim/T); only
  ln(diag(es)) is extracted for the per-token log(pos) subtraction.

  Masked columns contribute exp(0)=1 to every row sum: subtract
  K0 = 2S - 2n. per_tok = log(Ng + pos) - pos_sim/T; masked mean over
  2n tokens. Each core writes per_sample/8; the host sums the 8 cores.
"""

import sys

for _p in ("/opt/trn_rl_repo", "/opt/pypackages"):
    if _p not in sys.path:
        sys.path.append(_p)

from contextlib import ExitStack

import numpy as np

import bass_rust

import concourse.bass as bass
import concourse.tile as tile
from concourse import mybir
from concourse.bass_types import AP
from concourse.bass_utils import run_bass_kernel_spmd
from concourse.vector_clock import ScopedClock

# The walrus build in this container encodes at most 2 sync waits per
# instruction (bass_rust's inst_waits_full agrees), but Tile's semaphore
# assignment can attach more. Hoist excess waits onto unfusable same-engine
# NoOps immediately before the instruction — the engine executes its queue
# in order, so semantics are preserved.
_MAX_WAITS = 1


def _split_excess_waits(nc, ordered):
    for bb_name, insts in ordered.items():
        out = []
        changed = False
        for inst in insts:
            si = getattr(inst, "sync_info", None)
            waits = list(si.on_wait) if si is not None else []
            if len(waits) > _MAX_WAITS:
                changed = True
                extra, keep = waits[:-_MAX_WAITS], waits[-_MAX_WAITS:]
                for i in range(0, len(extra), _MAX_WAITS):
                    out.append(mybir.InstNoOp(
                        name=nc.get_next_instruction_name(),
                        sync_info=mybir.SyncInfo(
                            on_wait=extra[i:i + _MAX_WAITS], on_update=[]),
                        bass_nofuse=True,
                        engine=inst.engine,
                    ))
                si.on_wait = keep
            out.append(inst)
        if changed:
            insts[:] = out


_orig_lower_ordered_insts = tile.TileContext._lower_ordered_insts


def _patched_lower_ordered_insts(self, ordered):
    _split_excess_waits(self.nc, ordered)
    return _orig_lower_ordered_insts(self, ordered)


tile.TileContext._lower_ordered_insts = _patched_lower_ordered_insts


def _split_waits_drain_and_barrier(self, tick_clock, wait_clock):
    nc = self.nc
    probe = nc.sync.nop(nofuse=True)
    wait_clock.add_sem_waits(
        probe.ins, ScopedClock({None: tick_clock.global_clock}))
    si = probe.ins.sync_info
    waits = list(si.on_wait) if si is not None else []
    if len(waits) > _MAX_WAITS:
        si.on_wait = waits[:_MAX_WAITS]
        for i in range(_MAX_WAITS, len(waits), _MAX_WAITS):
            nxt = nc.sync.nop(nofuse=True)
            nxt.ins.sync_info = bass_rust.SyncInfo(
                on_wait=waits[i:i + _MAX_WAITS], on_update=[])
    nc.sync.drain()
    nc.all_engine_barrier()
    assert self.sems is not None
    popped = nc._tile_sem_poison_stack.pop()
    assert popped is self._sem_poison
    nc.clear_and_free_semaphores(list(self.sems.allocated().values()))
    nc.all_engine_barrier()


tile.TileContext._drain_and_barrier = _split_waits_drain_and_barrier

S, D, NCORES = 1536, 1024, 8
ST = S // 128            # 12 s-tiles per view
NB = 2 * ST              # 24 block rows of F
NCS = 2 * S // 512       # 6 column strips
KT = D // 128            # 8 contraction tiles
TEMP_INV = 20.0          # 1 / 0.05
FP8_SCALE = 8.0          # f entries ~N(0, 1/32); x8 keeps them in e4m3's
                         # normal range (|f|*8 <~ 2, well under 240)
F32 = mybir.dt.float32
BF16 = mybir.dt.bfloat16
FP8 = mybir.dt.float8e4
AF = mybir.ActivationFunctionType
ALU = mybir.AluOpType


def _build(num_devices: int = NCORES, debug_dump: bool = False) -> bass.Bass:
    nc = bass.Bass(num_devices=num_devices)
    h1T = nc.dram_tensor("h1T", [D, S], BF16, kind="ExternalInput")
    h2T = nc.dram_tensor("h2T", [D, S], BF16, kind="ExternalInput")
    # mask, pre-laid-out host-side as [128, ST] so token t = 128*col + row
    maskT = nc.dram_tensor("maskT", [128, ST], F32, kind="ExternalInput")
    # 8 * mask broadcast along partitions, token-row layout [128, S]
    m8bD = nc.dram_tensor("m8b", [128, S], BF16, kind="ExternalInput")
    out = nc.dram_tensor("loss", [1, 1], F32, kind="ExternalOutput")
    if debug_dump:
        ng_dump = nc.dram_tensor("ng_dump", [128, NB], F32,
                                 kind="ExternalOutput")
        cacc_dump = nc.dram_tensor("cacc_dump", [128, ST], F32,
                                   kind="ExternalOutput")

    exp_scale = TEMP_INV / (FP8_SCALE * FP8_SCALE)

    with tile.TileContext(nc) as tc, ExitStack() as ctx:
        const_pool = ctx.enter_context(tc.tile_pool(name="const", bufs=1))
        big = ctx.enter_context(tc.tile_pool(name="big", bufs=1))
        stat = ctx.enter_context(tc.tile_pool(name="stat", bufs=1))

        ones_col = const_pool.tile([128, 1], F32)
        nc.gpsimd.memset(ones_col[:], 1.0)
        ones_sq = const_pool.tile([128, 128], F32)
        nc.gpsimd.memset(ones_sq[:], 1.0)
        ones_bf = const_pool.tile([128, 1], BF16)
        nc.gpsimd.memset(ones_bf[:], 1.0)
        ones_r1 = const_pool.tile([1, 128], F32)
        nc.gpsimd.memset(ones_r1[:], 1.0)
        msk = const_pool.tile([128, ST], F32)
        nc.sync.dma_start(msk[:], maskT[:])
        m8b = const_pool.tile([128, S], BF16)
        nc.sync.dma_start(m8b[:], m8bD[:])

        hT1 = big.tile([128, KT, S], BF16)       # raw h1^T (bf16)
        hT2 = big.tile([128, KT, S], BF16)       # raw h2^T (bf16)
        fT1 = big.tile([128, KT, S], FP8)        # f1^T * 8, fp8e4
        fT2 = big.tile([128, KT, S], FP8)        # f2^T * 8
        acc = stat.tile([128, NB, NCS], F32)     # per-strip row sums
        cacc = stat.tile([128, ST], F32)         # B column sums (view-2 Ng)
        caccA = stat.tile([128, ST], F32)        # A col sums (skipped rows)
        caccD = stat.tile([128, ST], F32)        # D col sums (skipped rows)
        msk24 = stat.tile([128, NB], F32)
        pose_s = stat.tile([128, ST], F32)       # diag(es) = exp(pos_sim/T)
        negK0 = stat.tile([128, 1], F32)
        recn = stat.tile([1, 1], F32)

        # sub-triangle strips are never written; zero everything
        nc.gpsimd.memset(acc[:], 0.0)
        nc.gpsimd.memset(caccA[:], 0.0)
        nc.gpsimd.memset(caccD[:], 0.0)

        # input loads up front so the DMA queue streams continuously
        for k in range(KT):
            nc.sync.dma_start(hT1[:, k, :], h1T[k * 128:(k + 1) * 128, :])
        for k in range(KT):
            nc.sync.dma_start(hT2[:, k, :], h2T[k * 128:(k + 1) * 128, :])

        # ---- phase 0: mask-only precomputes ----
        with tc.tile_pool(name="ep0", bufs=1) as ep0, \
             tc.tile_pool(name="ep0_ps", bufs=1, space="PSUM") as ep0p:
            msum = ep0.tile([128, 1], F32)
            nc.vector.tensor_reduce(msum[:], msk[:],
                                    axis=mybir.AxisListType.X, op=ALU.add)
            nps = ep0p.tile([128, 1], F32)
            nc.tensor.matmul(nps[:], ones_sq[:], msum[:], start=True,
                             stop=True)
            # -K0 = 2n - 2S
            nc.scalar.activation(negK0[:], nps[:], AF.Copy, scale=2.0,
                                 bias=float(-2 * S))
            n2c = ep0.tile([1, 1], F32)
            nc.scalar.activation(n2c[:], nps[0:1, :], AF.Copy,
                                 scale=float(2 * num_devices))
            nc.vector.reciprocal(recn[:], n2c[:])   # 1/(2 n ncores)
            nc.vector.tensor_copy(msk24[:, 0:ST], msk[:])
            nc.vector.tensor_copy(msk24[:, ST:NB], msk[:])

        def make_prep(sqp, sbp, npp, hT, fT, tag, sq_scalar):
            """Chunked view prep: square -> norm -> rsqrt -> broadcast ->
            fq, per 512-token chunk so strip columns unblock early.
            sq_scalar: run the squares on ScalarE (else DVE).
            Returns (state, steps-dict of emitters)."""
            st = {}

            def e_sq(k):
                def f():
                    if "sq" not in st:
                        st["sq"] = sqp.tile([128, KT, S], BF16, tag="sq",
                                            name=f"sq_{tag}")
                    if sq_scalar:
                        nc.scalar.activation(st["sq"][:, k, :], hT[:, k, :],
                                             AF.Square)
                    else:
                        nc.vector.tensor_mul(st["sq"][:, k, :], hT[:, k, :],
                                             hT[:, k, :])
                return f

            def e_norm(c):
                def f():
                    if "pp" not in st:
                        st["pp"] = npp.tile([128, 3 * 512], F32, tag="nr",
                                            name=f"pp_{tag}")
                    pp = st["pp"]
                    for k in range(KT):
                        nc.tensor.matmul(
                            pp[0:1, c * 512:(c + 1) * 512],
                            ones_bf[:],
                            st["sq"][:, k, c * 512:(c + 1) * 512],
                            start=(k == 0), stop=(k == KT - 1),
                            skip_group_check=True)
                return f

            def e_schain(c):
                def f():
                    # s = exp(-0.5 ln |h|^2); then broadcast partition 0
                    # via a ones[128,1] x srow[1,512] outer product into
                    # the same PSUM chunk (nrow value already consumed),
                    # and fold in 8*mask.
                    pp = st["pp"]
                    lnn = sbp.tile([1, 512], F32, tag="lnn",
                                   name=f"lnn_{tag}_{c}")
                    nc.scalar.activation(lnn[:],
                                         pp[0:1, c * 512:(c + 1) * 512],
                                         AF.Ln)
                    srow = sbp.tile([1, 512], F32, tag="srow",
                                    name=f"srow_{tag}_{c}")
                    nc.scalar.activation(srow[:], lnn[:], AF.Exp,
                                         scale=-0.5)
                    nc.tensor.matmul(pp[:, c * 512:(c + 1) * 512],
                                     ones_r1[:], srow[:],
                                     start=True, stop=True,
                                     skip_group_check=True)
                    smask = sbp.tile([128, 512], BF16, tag="smask",
                                     name=f"smask_{tag}_{c}")
                    nc.vector.tensor_mul(smask[:],
                                         pp[:, c * 512:(c + 1) * 512],
                                         m8b[:, c * 512:(c + 1) * 512])
                    st[f"sm{c}"] = smask
                return f

            def e_fq(c, ks):
                def f():
                    for k in ks:
                        nc.vector.tensor_mul(
                            fT[:, k, c * 512:(c + 1) * 512],
                            hT[:, k, c * 512:(c + 1) * 512],
                            st[f"sm{c}"][:])
                return f

            return st, {"sq": e_sq, "norm": e_norm, "schain": e_schain,
                        "fq": e_fq}

        def strip(mmp, esp, scr, cs, r):
            """One [128,512] sim strip: matmuls, exp, row-sum into acc."""
            lhsT = fT1 if r < ST else fT2
            rT = r % ST
            rhsT = fT1 if cs < NCS // 2 else fT2
            csT = cs % (NCS // 2)
            ps = mmp.tile([128, 512], F32, tag="ps", name=f"ps_{cs}_{r}")
            for g in range(KT // 2):
                nc.tensor.matmul(
                    ps[:],
                    lhsT[:, 2 * g:2 * g + 2, rT * 128:(rT + 1) * 128],
                    rhsT[:, 2 * g:2 * g + 2, csT * 512:(csT + 1) * 512],
                    perf_mode=mybir.MatmulPerfMode.DoubleRow,
                    start=(g == 0), stop=(g == KT // 2 - 1))
            es = esp.tile([128, 512], BF16, tag="es", name=f"es_{cs}_{r}")
            # self-similarity diagonal block (exp -> e^20: must be zeroed
            # before the row sum). Only when row and column block coincide.
            self_bad = cs * 4 <= r < cs * 4 + 4
            # positive-counterpart diagonal (B quadrant): keep it in the
            # row sum (it IS exp(pos_sim/T), the "+pos" of the denom), but
            # extract the diagonal for the log(pos) term.
            pos_bc = r % ST + ST if r < ST else None
            pos_jb = None
            if pos_bc is not None and cs * 4 <= pos_bc < cs * 4 + 4:
                pos_jb = pos_bc - cs * 4
            if self_bad:
                jb = r - cs * 4
                nc.scalar.activation(es[:], ps[:], AF.Exp, scale=exp_scale)
                blk = es[:, jb * 128:(jb + 1) * 128]
                nc.gpsimd.affine_select(
                    out=blk, in_=blk, compare_op=ALU.not_equal,
                    fill=0.0, base=0, pattern=[[-1, 128]],
                    channel_multiplier=1)
                nc.vector.tensor_reduce(acc[:, r, cs:cs + 1], es[:],
                                        axis=mybir.AxisListType.X,
                                        op=ALU.add)
            else:
                nc.scalar.activation(es[:], ps[:], AF.Exp, scale=exp_scale,
                                     accum_out=acc[:, r, cs:cs + 1])
                if pos_jb is not None:
                    blk = es[:, pos_jb * 128:(pos_jb + 1) * 128]
                    dtmp = scr.tile([128, 128], BF16, tag="dtmp",
                                    name=f"dtmp_{cs}_{r}")
                    nc.gpsimd.affine_select(
                        out=dtmp[:], in_=blk, compare_op=ALU.is_equal,
                        fill=0.0, base=0, pattern=[[-1, 128]],
                        channel_multiplier=1)
                    nc.vector.tensor_reduce(pose_s[:, rT:rT + 1], dtmp[:],
                                            axis=mybir.AxisListType.X,
                                            op=ALU.add)
            return es

        with tc.tile_pool(name="mm_ps", bufs=4, space="PSUM") as mmp, \
             tc.tile_pool(name="nr_ps", bufs=1, space="PSUM") as npp, \
             tc.tile_pool(name="es", bufs=6) as esp, \
             tc.tile_pool(name="scr", bufs=3) as scr, \
             tc.tile_pool(name="sq", bufs=1) as sqp, \
             tc.tile_pool(name="sb", bufs=2) as sbp:

            # ---- phase A: view-1 prep (chunk c emitted fully before
            # c+1 so cs=0 strips can start after chunk 0) ----
            _sta, pa = make_prep(sqp, sbp, npp, hT1, fT1, "a",
                                 sq_scalar=False)
            _stb, pb = make_prep(sqp, sbp, npp, hT2, fT2, "b",
                                 sq_scalar=True)
            for k in range(KT):
                pa["sq"](k)()
            for c in range(3):
                pa["norm"](c)()
                pa["schain"](c)()
                pa["fq"](c, range(0, 4))()
                pa["fq"](c, range(4, 8))()
                # view-2 squares ride the ScalarE idle window between the
                # view-1 s-chain chunks (their DMAs land here too)
                if c > 0:
                    pb["sq"](2 * (c - 1))()
                    pb["sq"](2 * c - 1)()

            # ---- rest of view-2 prep interleaved into A' (remaining
            # squares with the first A' exps; DVE fq chunks under A's
            # tail; B' strips unblock per-chunk via region deps) ----
            inject = {}
            for k in range(4, KT):
                inject[1 + (k - 4)] = pb["sq"](k)
            inject[9] = pb["norm"](0)
            inject[10] = pb["schain"](0)
            inject[11] = pb["fq"](0, range(0, 4))
            inject[12] = pb["fq"](0, range(4, 8))
            inject[13] = pb["norm"](1)
            inject[14] = pb["schain"](1)
            inject[15] = pb["fq"](1, range(0, 4))
            inject[16] = pb["fq"](1, range(4, 8))
            inject[17] = pb["norm"](2)
            inject[18] = pb["schain"](2)
            inject[19] = pb["fq"](2, range(0, 4))
            inject[20] = pb["fq"](2, range(4, 8))
            _sno = [0]

            # ---- phase A': A-quadrant upper-triangle strips ----
            # A = f1 f1^T is symmetric: compute rows r <= 4cs+3 only. The
            # skipped blocks' row sums equal column sums of their mirrors,
            # accumulated over rows r < 4cs into caccA.
            with tc.tile_pool(name="cbA_ps", bufs=1, space="PSUM") as cbpA:
                for cs in range(NCS // 2):
                    ncr = 4 * cs            # colsum rows (above diag square)
                    cbtA = cbpA.tile([128, 4], F32, tag="cbA",
                                     name=f"cbtA_{cs}") if ncr else None
                    pcbA = ([cbtA[:, jb:jb + 1] for jb in range(4)]
                            if ncr else None)
                    for r in range(4 * cs + 4):
                        es = strip(mmp, esp, scr, cs, r)
                        _sno[0] += 1
                        if _sno[0] in inject:
                            inject[_sno[0]]()
                        if r < ncr:
                            for jb in range(4):
                                nc.tensor.matmul(
                                    pcbA[jb],
                                    es[:, jb * 128:(jb + 1) * 128],
                                    ones_bf[:],
                                    start=(r == 0 and jb == 0),
                                    stop=(r == ncr - 1),
                                    skip_group_check=True)
                    if ncr:
                        for jb in range(4):
                            nc.vector.tensor_copy(
                                caccA[:, 4 * cs + jb:4 * cs + jb + 1],
                                pcbA[jb])

            # ---- phase B': B strips (full) + D upper-triangle strips ----
            # B needs its column sums (= C-quadrant row sums) over all 12
            # rows. D = f2 f2^T is symmetric like A: rows r-ST <= 4csq+3,
            # with column sums over D rows above the diag square -> caccD.
            with tc.tile_pool(name="cb_ps", bufs=1, space="PSUM") as cbp:
                for cs in range(NCS // 2, NCS):
                    csq = cs - NCS // 2
                    ndc = 4 * csq           # D colsum rows (above square)
                    # one PSUM bank holds both groups; the B group's
                    # accumulation fully precedes the D group's start, so
                    # the bank-wide has_written clear is harmless.
                    cbt = cbp.tile([128, 8], F32, tag="cb",
                                   name=f"cbt_{cs}")
                    pcb = [cbt[:, jb:jb + 1] for jb in range(4)]
                    pcbD = ([cbt[:, 4 + jb:5 + jb] for jb in range(4)]
                            if ndc else None)
                    for r in range(ST + 4 * csq + 4):
                        es = strip(mmp, esp, scr, cs, r)
                        if r < ST:
                            for jb in range(4):
                                nc.tensor.matmul(
                                    pcb[jb],
                                    es[:, jb * 128:(jb + 1) * 128],
                                    ones_bf[:],
                                    start=(r == 0 and jb == 0),
                                    stop=(r == ST - 1),
                                    skip_group_check=True)
                        elif r - ST < ndc:
                            for jb in range(4):
                                nc.tensor.matmul(
                                    pcbD[jb],
                                    es[:, jb * 128:(jb + 1) * 128],
                                    ones_bf[:],
                                    start=(r == ST and jb == 0),
                                    stop=(r == ST + ndc - 1),
                                    skip_group_check=True)
                    c0 = csq * 4
                    for jb in range(4):
                        nc.vector.tensor_copy(cacc[:, c0 + jb:c0 + jb + 1],
                                              pcb[jb])
                    if ndc:
                        for jb in range(4):
                            nc.vector.tensor_copy(
                                caccD[:, c0 + jb:c0 + jb + 1],
                                pcbD[jb])

        # ---- phase C: final reduction chain ----
        with tc.tile_pool(name="ep", bufs=1) as ep, \
             tc.tile_pool(name="ep_ps", bufs=1, space="PSUM") as epp:
            ng = ep.tile([128, NB], F32)
            nc.vector.tensor_reduce(ng[:], acc[:], axis=mybir.AxisListType.X,
                                    op=ALU.add)
            nc.vector.tensor_add(ng[:, 0:ST], ng[:, 0:ST], caccA[:])
            nc.vector.tensor_add(ng[:, ST:NB], ng[:, ST:NB], cacc[:])
            nc.vector.tensor_add(ng[:, ST:NB], ng[:, ST:NB], caccD[:])
            if debug_dump:
                nc.sync.dma_start(ng_dump[:], ng[:])
                nc.sync.dma_start(cacc_dump[:], cacc[:])
            # denom = Ng + pos: the positive-pair diagonal was left in the
            # row sums, so only the masked-column surplus K0 is removed.
            denom = ep.tile([128, NB], F32)
            nc.vector.tensor_scalar_add(denom[:], ng[:], negK0[:])
            lg = ep.tile([128, NB], F32)
            nc.scalar.activation(lg[:], denom[:], AF.Ln)
            # pos_sim/T = ln(diag(es)); mask and double it
            lnp = ep.tile([128, ST], F32)
            nc.scalar.activation(lnp[:], pose_s[:], AF.Ln)
            poss20m = ep.tile([128, NB], F32)
            nc.vector.tensor_mul(poss20m[:, 0:ST], lnp[:], msk[:])
            nc.vector.tensor_copy(poss20m[:, ST:NB], poss20m[:, 0:ST])
            ptok = ep.tile([128, NB], F32)
            nc.vector.tensor_mul(ptok[:], lg[:], msk24[:])
            nc.vector.tensor_sub(ptok[:], ptok[:], poss20m[:])
            tsum = ep.tile([128, 1], F32)
            nc.vector.tensor_reduce(tsum[:], ptok[:],
                                    axis=mybir.AxisListType.X, op=ALU.add)
            lps = epp.tile([1, 1], F32)
            nc.tensor.matmul(lps[:], ones_col[:], tsum[:], start=True,
                             stop=True)
            lsb = ep.tile([1, 1], F32)
            nc.vector.tensor_mul(lsb[:], lps[:], recn[:])
            # per-core partial (per_sample / ncores); host sums the cores
            nc.sync.dma_start(out[:], lsb[:])

    return nc


_NC = None


def _mask_layout(mask_row: np.ndarray) -> np.ndarray:
    # token t = 128 * col + row  ->  [128, ST]
    return np.ascontiguousarray(
        mask_row.astype(np.float32).reshape(ST, 128).T)


def _in_map(h1_b: np.ndarray, h2_b: np.ndarray, mask_b: np.ndarray) -> dict:
    import ml_dtypes
    bf16 = ml_dtypes.bfloat16
    m8 = (mask_b.astype(np.float32) * np.float32(FP8_SCALE)).astype(bf16)
    return {
        "h1T": np.ascontiguousarray(h1_b.T.astype(bf16)),
        "h2T": np.ascontiguousarray(h2_b.T.astype(bf16)),
        "maskT": _mask_layout(mask_b),
        "m8b": np.ascontiguousarray(
            np.broadcast_to(m8[None, :], (128, S))),
    }


def kernel(last_hidden_states_1, last_hidden_states_2, token_mask_batch):
    global _NC
    h1 = np.ascontiguousarray(np.asarray(last_hidden_states_1,
                                         dtype=np.float32))
    h2 = np.ascontiguousarray(np.asarray(last_hidden_states_2,
                                         dtype=np.float32))
    mask = np.asarray(token_mask_batch)
    assert h1.shape == (NCORES, S, D), h1.shape

    if _NC is None:
        _NC = _build(NCORES)

    in_maps = [_in_map(h1[b], h2[b], mask[b]) for b in range(NCORES)]
    res = run_bass_kernel_spmd(_NC, in_maps, list(range(NCORES)))
    loss = np.float32(sum(
        float(np.asarray(res.results[b]["loss"]).reshape(()))
        for b in range(NCORES)))
    return loss


# revision 22
# speedup vs baseline: 1.0347x; 1.0347x over previous
"""ContraCLM token-level contrastive loss on 8 Trainium2 NeuronCores.

Data-parallel over the batch: core b handles sample b (B=8). Per core,
with S=1536, D=1024, T=0.05:

  The host supplies each view pre-transposed (hT = h.T, [D, S] fp32), a
  row-broadcast masked scale helper m8b = broadcast(8*mask) [128, S],
  and the token-major mask maskT [128, S/128]. On device, per view:

    sq   = hT * hT                      (GpSimd, bf16 out)
    nrow = ones^T @ sq                  (PE column sums -> |h_t|^2, [1,S])
    srow = exp(-0.5 ln nrow)            (ScalarE, natural_log_exp set)
    sb   = partition_broadcast(srow)    (GpSimd)
    smask= sb * m8b                     (DVE; 8/||h_t|| * mask)
    fT   = hT * smask                   (GpSimd, fp8e4 out, x8 scaled)

  No on-device transposes: fT is built directly in [D, 2S] layout.

  sim = F F^T as [128, 512] PSUM strips (fp8 DoubleRow, K=1024).
  exp(sim/T) row sums come free from the ScalarE activation accumulator.
  A and D quadrants are symmetric: only rows r <= 4cs+3 of each column
  strip are computed; the skipped blocks' row sums are recovered as
  PSUM-accumulated ones-matmul column sums of their mirrors (caccA/
  caccD), like the C quadrant reuses B's column sums (cacc).

  Self-similarity diagonal blocks are zeroed (affine_select + DVE row
  sum) before summing: exp(1/T) = e^20 would destroy the fp32 sum. The
  positive-counterpart diagonal (B quadrant) is LEFT IN the row sum:
  denom = Ng + pos and the included diagonal IS exp(pos_sim/T); only
  ln(diag(es)) is extracted for the per-token log(pos) subtraction.

  Masked columns contribute exp(0)=1 to every row sum: subtract
  K0 = 2S - 2n. per_tok = log(Ng + pos) - pos_sim/T; masked mean over
  2n tokens. Each core writes per_sample/8; the host sums the 8 cores.
"""

import sys

for _p in ("/opt/trn_rl_repo", "/opt/pypackages"):
    if _p not in sys.path:
        sys.path.append(_p)

from contextlib import ExitStack

import numpy as np

import bass_rust

import concourse.bass as bass
import concourse.tile as tile
from concourse import mybir
from concourse.bass_types import AP
from concourse.bass_utils import run_bass_kernel_spmd
from concourse.vector_clock import ScopedClock

# The walrus build in this container encodes at most 2 sync waits per
# instruction (bass_rust's inst_waits_full agrees), but Tile's semaphore
# assignment can attach more. Hoist excess waits onto unfusable same-engine
# NoOps immediately before the instruction — the engine executes its queue
# in order, so semantics are preserved.
_MAX_WAITS = 1


def _split_excess_waits(nc, ordered):
    for bb_name, insts in ordered.items():
        out = []
        changed = False
        for inst in insts:
            si = getattr(inst, "sync_info", None)
            waits = list(si.on_wait) if si is not None else []
            if len(waits) > _MAX_WAITS:
                changed = True
                extra, keep = waits[:-_MAX_WAITS], waits[-_MAX_WAITS:]
                for i in range(0, len(extra), _MAX_WAITS):
                    out.append(mybir.InstNoOp(
                        name=nc.get_next_instruction_name(),
                        sync_info=mybir.SyncInfo(
                            on_wait=extra[i:i + _MAX_WAITS], on_update=[]),
                        bass_nofuse=True,
                        engine=inst.engine,
                    ))
                si.on_wait = keep
            out.append(inst)
        if changed:
            insts[:] = out


_orig_lower_ordered_insts = tile.TileContext._lower_ordered_insts


def _patched_lower_ordered_insts(self, ordered):
    _split_excess_waits(self.nc, ordered)
    return _orig_lower_ordered_insts(self, ordered)


tile.TileContext._lower_ordered_insts = _patched_lower_ordered_insts


def _split_waits_drain_and_barrier(self, tick_clock, wait_clock):
    nc = self.nc
    probe = nc.sync.nop(nofuse=True)
    wait_clock.add_sem_waits(
        probe.ins, ScopedClock({None: tick_clock.global_clock}))
    si = probe.ins.sync_info
    waits = list(si.on_wait) if si is not None else []
    if len(waits) > _MAX_WAITS:
        si.on_wait = waits[:_MAX_WAITS]
        for i in range(_MAX_WAITS, len(waits), _MAX_WAITS):
            nxt = nc.sync.nop(nofuse=True)
            nxt.ins.sync_info = bass_rust.SyncInfo(
                on_wait=waits[i:i + _MAX_WAITS], on_update=[])
    nc.sync.drain()
    nc.all_engine_barrier()
    assert self.sems is not None
    popped = nc._tile_sem_poison_stack.pop()
    assert popped is self._sem_poison
    nc.clear_and_free_semaphores(list(self.sems.allocated().values()))
    nc.all_engine_barrier()


tile.TileContext._drain_and_barrier = _split_waits_drain_and_barrier

S, D, NCORES = 1536, 1024, 8
ST = S // 128            # 12 s-tiles per view
NB = 2 * ST              # 24 block rows of F
NCS = 2 * S // 512       # 6 column strips
KT = D // 128            # 8 contraction tiles
TEMP_INV = 20.0          # 1 / 0.05
FP8_SCALE = 8.0          # f entries ~N(0, 1/32); x8 keeps them in e4m3's
                         # normal range (|f|*8 <~ 2, well under 240)
F32 = mybir.dt.float32
BF16 = mybir.dt.bfloat16
FP8 = mybir.dt.float8e4
AF = mybir.ActivationFunctionType
ALU = mybir.AluOpType


def _build(num_devices: int = NCORES, debug_dump: bool = False) -> bass.Bass:
    nc = bass.Bass(num_devices=num_devices)
    h1T = nc.dram_tensor("h1T", [D, S], BF16, kind="ExternalInput")
    h2T = nc.dram_tensor("h2T", [D, S], BF16, kind="ExternalInput")
    # mask, pre-laid-out host-side as [128, ST] so token t = 128*col + row
    maskT = nc.dram_tensor("maskT", [128, ST], F32, kind="ExternalInput")
    # 8 * mask broadcast along partitions, token-row layout [128, S]
    m8bD = nc.dram_tensor("m8b", [128, S], BF16, kind="ExternalInput")
    out = nc.dram_tensor("loss", [1, 1], F32, kind="ExternalOutput")
    if debug_dump:
        ng_dump = nc.dram_tensor("ng_dump", [128, NB], F32,
                                 kind="ExternalOutput")
        cacc_dump = nc.dram_tensor("cacc_dump", [128, ST], F32,
                                   kind="ExternalOutput")

    exp_scale = TEMP_INV / (FP8_SCALE * FP8_SCALE)

    with tile.TileContext(nc) as tc, ExitStack() as ctx:
        const_pool = ctx.enter_context(tc.tile_pool(name="const", bufs=1))
        big = ctx.enter_context(tc.tile_pool(name="big", bufs=1))
        stat = ctx.enter_context(tc.tile_pool(name="stat", bufs=1))

        ones_col = const_pool.tile([128, 1], F32)
        nc.gpsimd.memset(ones_col[:], 1.0)
        ones_sq = const_pool.tile([128, 128], F32)
        nc.gpsimd.memset(ones_sq[:], 1.0)
        ones_bf = const_pool.tile([128, 1], BF16)
        nc.gpsimd.memset(ones_bf[:], 1.0)
        ones_r1 = const_pool.tile([1, 128], F32)
        nc.gpsimd.memset(ones_r1[:], 1.0)
        msk = const_pool.tile([128, ST], F32)
        nc.sync.dma_start(msk[:], maskT[:])
        m8b = const_pool.tile([128, S], BF16)
        nc.sync.dma_start(m8b[:], m8bD[:])

        hT1 = big.tile([128, KT, S], BF16)       # raw h1^T (bf16)
        hT2 = big.tile([128, KT, S], BF16)       # raw h2^T (bf16)
        fT1 = big.tile([128, KT, S], FP8)        # f1^T * 8, fp8e4
        fT2 = big.tile([128, KT, S], FP8)        # f2^T * 8
        acc = stat.tile([128, NB, NCS], F32)     # per-strip row sums
        cacc = stat.tile([128, ST], F32)         # B column sums (view-2 Ng)
        caccA = stat.tile([128, ST], F32)        # A col sums (skipped rows)
        caccD = stat.tile([128, ST], F32)        # D col sums (skipped rows)
        msk24 = stat.tile([128, NB], F32)
        pose_s = stat.tile([128, ST], F32)       # diag(es) = exp(pos_sim/T)
        negK0 = stat.tile([128, 1], F32)
        recn = stat.tile([1, 1], F32)

        # sub-triangle strips are never written; zero everything
        nc.gpsimd.memset(acc[:], 0.0)
        nc.gpsimd.memset(caccA[:], 0.0)
        nc.gpsimd.memset(caccD[:], 0.0)

        # input loads up front so the DMA queue streams continuously
        for k in range(KT):
            nc.sync.dma_start(hT1[:, k, :], h1T[k * 128:(k + 1) * 128, :])
        for k in range(KT):
            nc.sync.dma_start(hT2[:, k, :], h2T[k * 128:(k + 1) * 128, :])

        # ---- phase 0: mask-only precomputes ----
        with tc.tile_pool(name="ep0", bufs=1) as ep0, \
             tc.tile_pool(name="ep0_ps", bufs=1, space="PSUM") as ep0p:
            msum = ep0.tile([128, 1], F32)
            nc.vector.tensor_reduce(msum[:], msk[:],
                                    axis=mybir.AxisListType.X, op=ALU.add)
            nps = ep0p.tile([128, 1], F32)
            nc.tensor.matmul(nps[:], ones_sq[:], msum[:], start=True,
                             stop=True)
            # -K0 = 2n - 2S
            nc.scalar.activation(negK0[:], nps[:], AF.Copy, scale=2.0,
                                 bias=float(-2 * S))
            n2c = ep0.tile([1, 1], F32)
            nc.scalar.activation(n2c[:], nps[0:1, :], AF.Copy,
                                 scale=float(2 * num_devices))
            nc.vector.reciprocal(recn[:], n2c[:])   # 1/(2 n ncores)
            nc.vector.tensor_copy(msk24[:, 0:ST], msk[:])
            nc.vector.tensor_copy(msk24[:, ST:NB], msk[:])

        def make_prep(sqp, sbp, npp, hT, fT, tag, sq_scalar):
            """Chunked view prep: square -> norm -> rsqrt -> broadcast ->
            fq, per 512-token chunk so strip columns unblock early.
            sq_scalar: run the squares on ScalarE (else DVE).
            Returns (state, steps-dict of emitters)."""
            st = {}

            def e_sq(k):
                def f():
                    if "sq" not in st:
                        st["sq"] = sqp.tile([128, KT, S], BF16, tag="sq",
                                            name=f"sq_{tag}")
                    if sq_scalar:
                        nc.scalar.activation(st["sq"][:, k, :], hT[:, k, :],
                                             AF.Square)
                    else:
                        nc.vector.tensor_mul(st["sq"][:, k, :], hT[:, k, :],
                                             hT[:, k, :])
                return f

            def e_norm(c):
                def f():
                    if "pp" not in st:
                        st["pp"] = npp.tile([128, 3 * 512], F32, tag="nr",
                                            name=f"pp_{tag}")
                    pp = st["pp"]
                    for k in range(KT):
                        nc.tensor.matmul(
                            pp[0:1, c * 512:(c + 1) * 512],
                            ones_bf[:],
                            st["sq"][:, k, c * 512:(c + 1) * 512],
                            start=(k == 0), stop=(k == KT - 1),
                            skip_group_check=True)
                return f

            def e_schain(c):
                def f():
                    # s = exp(-0.5 ln |h|^2); then broadcast partition 0
                    # via a ones[128,1] x srow[1,512] outer product into
                    # the same PSUM chunk (nrow value already consumed),
                    # and fold in 8*mask.
                    pp = st["pp"]
                    lnn = sbp.tile([1, 512], F32, tag="lnn",
                                   name=f"lnn_{tag}_{c}")
                    nc.scalar.activation(lnn[:],
                                         pp[0:1, c * 512:(c + 1) * 512],
                                         AF.Ln)
                    srow = sbp.tile([1, 512], F32, tag="srow",
                                    name=f"srow_{tag}_{c}")
                    nc.scalar.activation(srow[:], lnn[:], AF.Exp,
                                         scale=-0.5)
                    nc.tensor.matmul(pp[:, c * 512:(c + 1) * 512],
                                     ones_r1[:], srow[:],
                                     start=True, stop=True,
                                     skip_group_check=True)
                    smask = sbp.tile([128, 512], BF16, tag="smask",
                                     name=f"smask_{tag}_{c}")
                    nc.vector.tensor_mul(smask[:],
                                         pp[:, c * 512:(c + 1) * 512],
                                         m8b[:, c * 512:(c + 1) * 512])
                    st[f"sm{c}"] = smask
                return f

            def e_fq(c, ks):
                def f():
                    for k in ks:
                        nc.vector.tensor_mul(
                            fT[:, k, c * 512:(c + 1) * 512],
                            hT[:, k, c * 512:(c + 1) * 512],
                            st[f"sm{c}"][:])
                return f

            return st, {"sq": e_sq, "norm": e_norm, "schain": e_schain,
                        "fq": e_fq}

        def strip(mmp, esp, scr, cs, r):
            """One [128,512] sim strip: matmuls, exp, row-sum into acc."""
            lhsT = fT1 if r < ST else fT2
            rT = r % ST
            rhsT = fT1 if cs < NCS // 2 else fT2
            csT = cs % (NCS // 2)
            ps = mmp.tile([128, 512], F32, tag="ps", name=f"ps_{cs}_{r}")
            for g in range(KT // 2):
                nc.tensor.matmul(
                    ps[:],
                    lhsT[:, 2 * g:2 * g + 2, rT * 128:(rT + 1) * 128],
                    rhsT[:, 2 * g:2 * g + 2, csT * 512:(csT + 1) * 512],
                    perf_mode=mybir.MatmulPerfMode.DoubleRow,
                    start=(g == 0), stop=(g == KT // 2 - 1))
            es = esp.tile([128, 512], BF16, tag="es", name=f"es_{cs}_{r}")
            # self-similarity diagonal block (exp -> e^20: must be zeroed
            # before the row sum). Only when row and column block coincide.
            self_bad = cs * 4 <= r < cs * 4 + 4
            # positive-counterpart diagonal (B quadrant): keep it in the
            # row sum (it IS exp(pos_sim/T), the "+pos" of the denom), but
            # extract the diagonal for the log(pos) term.
            pos_bc = r % ST + ST if r < ST else None
            pos_jb = None
            if pos_bc is not None and cs * 4 <= pos_bc < cs * 4 + 4:
                pos_jb = pos_bc - cs * 4
            if self_bad:
                jb = r - cs * 4
                nc.scalar.activation(es[:], ps[:], AF.Exp, scale=exp_scale)
                blk = es[:, jb * 128:(jb + 1) * 128]
                nc.gpsimd.affine_select(
                    out=blk, in_=blk, compare_op=ALU.not_equal,
                    fill=0.0, base=0, pattern=[[-1, 128]],
                    channel_multiplier=1)
                nc.vector.tensor_reduce(acc[:, r, cs:cs + 1], es[:],
                                        axis=mybir.AxisListType.X,
                                        op=ALU.add)
            else:
                nc.scalar.activation(es[:], ps[:], AF.Exp, scale=exp_scale,
                                     accum_out=acc[:, r, cs:cs + 1])
                if pos_jb is not None:
                    blk = es[:, pos_jb * 128:(pos_jb + 1) * 128]
                    dtmp = scr.tile([128, 128], BF16, tag="dtmp",
                                    name=f"dtmp_{cs}_{r}")
                    nc.gpsimd.affine_select(
                        out=dtmp[:], in_=blk, compare_op=ALU.is_equal,
                        fill=0.0, base=0, pattern=[[-1, 128]],
                        channel_multiplier=1)
                    nc.vector.tensor_reduce(pose_s[:, rT:rT + 1], dtmp[:],
                                            axis=mybir.AxisListType.X,
                                            op=ALU.add)
            return es

        with tc.tile_pool(name="mm_ps", bufs=4, space="PSUM") as mmp, \
             tc.tile_pool(name="nr_ps", bufs=1, space="PSUM") as npp, \
             tc.tile_pool(name="es", bufs=6) as esp, \
             tc.tile_pool(name="scr", bufs=3) as scr, \
             tc.tile_pool(name="sq", bufs=1) as sqp, \
             tc.tile_pool(name="sb", bufs=2) as sbp:

            # ---- phase A: view-1 prep (chunk c emitted fully before
            # c+1 so cs=0 strips can start after chunk 0) ----
            _sta, pa = make_prep(sqp, sbp, npp, hT1, fT1, "a",
                                 sq_scalar=False)
            for k in range(KT):
                pa["sq"](k)()
            for c in range(3):
                pa["norm"](c)()
                pa["schain"](c)()
                pa["fq"](c, range(0, 4))()
                pa["fq"](c, range(4, 8))()

            # ---- view-2 prep, interleaved into A' (ScalarE squares ride
            # along with the A' exps; DVE fq chunks run under A's tail;
            # B' strips unblock per-chunk via region deps) ----
            _stb, pb = make_prep(sqp, sbp, npp, hT2, fT2, "b",
                                 sq_scalar=True)
            inject = {}
            for k in range(KT):
                inject[1 + k] = pb["sq"](k)
            inject[9] = pb["norm"](0)
            inject[10] = pb["schain"](0)
            inject[11] = pb["fq"](0, range(0, 4))
            inject[12] = pb["fq"](0, range(4, 8))
            inject[13] = pb["norm"](1)
            inject[14] = pb["schain"](1)
            inject[15] = pb["fq"](1, range(0, 4))
            inject[16] = pb["fq"](1, range(4, 8))
            inject[17] = pb["norm"](2)
            inject[18] = pb["schain"](2)
            inject[19] = pb["fq"](2, range(0, 4))
            inject[20] = pb["fq"](2, range(4, 8))
            _sno = [0]

            # ---- phase A': A-quadrant upper-triangle strips ----
            # A = f1 f1^T is symmetric: compute rows r <= 4cs+3 only. The
            # skipped blocks' row sums equal column sums of their mirrors,
            # accumulated over rows r < 4cs into caccA.
            with tc.tile_pool(name="cbA_ps", bufs=1, space="PSUM") as cbpA:
                for cs in range(NCS // 2):
                    ncr = 4 * cs            # colsum rows (above diag square)
                    cbtA = cbpA.tile([128, 4], F32, tag="cbA",
                                     name=f"cbtA_{cs}") if ncr else None
                    pcbA = ([cbtA[:, jb:jb + 1] for jb in range(4)]
                            if ncr else None)
                    for r in range(4 * cs + 4):
                        es = strip(mmp, esp, scr, cs, r)
                        _sno[0] += 1
                        if _sno[0] in inject:
                            inject[_sno[0]]()
                        if r < ncr:
                            for jb in range(4):
                                nc.tensor.matmul(
                                    pcbA[jb],
                                    es[:, jb * 128:(jb + 1) * 128],
                                    ones_bf[:],
                                    start=(r == 0 and jb == 0),
                                    stop=(r == ncr - 1),
                                    skip_group_check=True)
                    if ncr:
                        for jb in range(4):
                            nc.vector.tensor_copy(
                                caccA[:, 4 * cs + jb:4 * cs + jb + 1],
                                pcbA[jb])

            # ---- phase B': B strips (full) + D upper-triangle strips ----
            # B needs its column sums (= C-quadrant row sums) over all 12
            # rows. D = f2 f2^T is symmetric like A: rows r-ST <= 4csq+3,
            # with column sums over D rows above the diag square -> caccD.
            with tc.tile_pool(name="cb_ps", bufs=1, space="PSUM") as cbp:
                for cs in range(NCS // 2, NCS):
                    csq = cs - NCS // 2
                    ndc = 4 * csq           # D colsum rows (above square)
                    # one PSUM bank holds both groups; the B group's
                    # accumulation fully precedes the D group's start, so
                    # the bank-wide has_written clear is harmless.
                    cbt = cbp.tile([128, 8], F32, tag="cb",
                                   name=f"cbt_{cs}")
                    pcb = [cbt[:, jb:jb + 1] for jb in range(4)]
                    pcbD = ([cbt[:, 4 + jb:5 + jb] for jb in range(4)]
                            if ndc else None)
                    for r in range(ST + 4 * csq + 4):
                        es = strip(mmp, esp, scr, cs, r)
                        if r < ST:
                            for jb in range(4):
                                nc.tensor.matmul(
                                    pcb[jb],
                                    es[:, jb * 128:(jb + 1) * 128],
                                    ones_bf[:],
                                    start=(r == 0 and jb == 0),
                                    stop=(r == ST - 1),
                                    skip_group_check=True)
                        elif r - ST < ndc:
                            for jb in range(4):
                                nc.tensor.matmul(
                                    pcbD[jb],
                                    es[:, jb * 128:(jb + 1) * 128],
                                    ones_bf[:],
                                    start=(r == ST and jb == 0),
                                    stop=(r == ST + ndc - 1),
                                    skip_group_check=True)
                    c0 = csq * 4
                    for jb in range(4):
                        nc.vector.tensor_copy(cacc[:, c0 + jb:c0 + jb + 1],
                                              pcb[jb])
                    if ndc:
                        for jb in range(4):
                            nc.vector.tensor_copy(
                                caccD[:, c0 + jb:c0 + jb + 1],
                                pcbD[jb])

        # ---- phase C: final reduction chain ----
        with tc.tile_pool(name="ep", bufs=1) as ep, \
             tc.tile_pool(name="ep_ps", bufs=1, space="PSUM") as epp:
            ng = ep.tile([128, NB], F32)
            nc.vector.tensor_reduce(ng[:], acc[:], axis=mybir.AxisListType.X,
                                    op=ALU.add)
            nc.vector.tensor_add(ng[:, 0:ST], ng[:, 0:ST], caccA[:])
            nc.vector.tensor_add(ng[:, ST:NB], ng[:, ST:NB], cacc[:])
            nc.vector.tensor_add(ng[:, ST:NB], ng[:, ST:NB], caccD[:])
            if debug_dump:
                nc.sync.dma_start(ng_dump[:], ng[:])
                nc.sync.dma_start(cacc_dump[:], cacc[:])
            # denom = Ng + pos: the positive-pair diagonal was left in the
            # row sums, so only the masked-column surplus K0 is removed.
            denom = ep.tile([128, NB], F32)
            nc.vector.tensor_scalar_add(denom[:], ng[:], negK0[:])
            lg = ep.tile([128, NB], F32)
            nc.scalar.activation(lg[:], denom[:], AF.Ln)
            # pos_sim/T = ln(diag(es)); mask and double it
            lnp = ep.tile([128, ST], F32)
            nc.scalar.activation(lnp[:], pose_s[:], AF.Ln)
            poss20m = ep.tile([128, NB], F32)
            nc.vector.tensor_mul(poss20m[:, 0:ST], lnp[:], msk[:])
            nc.vector.tensor_copy(poss20m[:, ST:NB], poss20m[:, 0:ST])
            ptok = ep.tile([128, NB], F32)
            nc.vector.tensor_mul(ptok[:], lg[:], msk24[:])
            nc.vector.tensor_sub(ptok[:], ptok[:], poss20m[:])
            tsum = ep.tile([128, 1], F32)
            nc.vector.tensor_reduce(tsum[:], ptok[:],
                                    axis=mybir.AxisListType.X, op=ALU.add)
            lps = epp.tile([1, 1], F32)
            nc.tensor.matmul(lps[:], ones_col[:], tsum[:], start=True,
                             stop=True)
            lsb = ep.tile([1, 1], F32)
            nc.vector.tensor_mul(lsb[:], lps[:], recn[:])
            # per-core partial (per_sample / ncores); host sums the cores
            nc.sync.dma_start(out[:], lsb[:])

    return nc


_NC = None


def _mask_layout(mask_row: np.ndarray) -> np.ndarray:
    # token t = 128 * col + row  ->  [128, ST]
    return np.ascontiguousarray(
        mask_row.astype(np.float32).reshape(ST, 128).T)


def _in_map(h1_b: np.ndarray, h2_b: np.ndarray, mask_b: np.ndarray) -> dict:
    import ml_dtypes
    bf16 = ml_dtypes.bfloat16
    m8 = (mask_b.astype(np.float32) * np.float32(FP8_SCALE)).astype(bf16)
    return {
        "h1T": np.ascontiguousarray(h1_b.T.astype(bf16)),
        "h2T": np.ascontiguousarray(h2_b.T.astype(bf16)),
        "maskT": _mask_layout(mask_b),
        "m8b": np.ascontiguousarray(
            np.broadcast_to(m8[None, :], (128, S))),
    }


def kernel(last_hidden_states_1, last_hidden_states_2, token_mask_batch):
    global _NC
    h1 = np.ascontiguousarray(np.asarray(last_hidden_states_1,
                                         dtype=np.float32))
    h2 = np.ascontiguousarray(np.asarray(last_hidden_states_2,
                                         dtype=np.float32))
    mask = np.asarray(token_mask_batch)
    assert h1.shape == (NCORES, S, D), h1.shape

    if _NC is None:
        _NC = _build(NCORES)

    in_maps = [_in_map(h1[b], h2[b], mask[b]) for b in range(NCORES)]
    res = run_bass_kernel_spmd(_NC, in_maps, list(range(NCORES)))
    loss = np.float32(sum(
        float(np.asarray(res.results[b]["loss"]).reshape(()))
        for b in range(NCORES)))
    return loss
